# revision 17
# baseline (speedup 1.0000x reference)
"""Trainium2 Bass kernel for LoRA linear: y = x @ (W + 2*B@A).T + b.

Full inputs: x (8, 2048, 2048) f32, W (2048, 2048) f32, b (2048,) f32,
B (2048, 16) f32, A (16, 2048) f32.  Output (8, 2048, 2048) f32.

Sharding: data-parallel over the batch dim — core i computes
y[i] = x[i] @ w.T + b with the merged weight w = W + 2*B@A.

Host-side layout prep (sharding/packing only, no math): inputs are
pre-transposed, pre-cast to bf16, and pre-tiled into the exact SBUF
layouts the device wants, so every DMA is 128 fat descriptors (HWDGE
descriptor generation was the load bottleneck at ~3ns/descriptor):
  xp[c, p, t, sc] = x[c*256+sc, t*128+p]   (s-chunk-major tiles)
  Wp[ob, p, t, oc] = W[ob*512+oc, t*128+p] (o-bank-major tiles)
  BTs = 2*B.T (exact power-of-two scale; bf16 values identical to a
  device-side scale), A cast to bf16.

Device schedule (per core), tuned from perfetto traces:
  - all loads on ONE HWDGE ring (sync) in consumption-priority order
    (the order IS the prefetch schedule); stores on the other (scalar).
  - A and 2B.T land in zero-memset [128, D] tiles so the rank-16 delta
    matmuls are full-K=128 matmuls — identical shape to the GEMM MMs
    (K is free on the PE; K=16 stationaries cost ~+100ns transitions).
  - bank-0 delta merges are two-phase (ACT evicts PSUM to a bf16
    staging tile, DVE adds all-bf16 at 2x rate) so the head merge wave
    is split across two engines instead of serialized on the DVE.
  - throwaway warm-up matmuls keep the PE activity monitor from
    re-throttling the clock during the DMA/DVE-paced head (HAM drops
    the PE to 1.2 GHz after ~3.4us of low activity density).
  - main GEMM is ob-major: per output bank, 16 row-tiles of 16
    accumulating [128,128]x[128,512] bf16 matmuls; DVE adds the bias
    during PSUM->SBUF eviction.  Delta matmuls for bank ob+1 are
    spread two-per-group through the second half of pass ob so the PE
    stream never develops idle clusters.
"""

import numpy as np
import ml_dtypes

import concourse.bacc as bacc
import concourse.mybir as mybir
import concourse.tile as tile
from concourse.bass_utils import run_bass_kernel_spmd

N_CORES = 8
BATCH, S, D = 8, 2048, 2048
RANK = 16
SCALE = 2.0  # alpha / rank = 32 / 16
P = 128  # partitions
FREE = 512  # f32 elems per PSUM bank
ND = D // P  # 16 contraction tiles
NS = S // P  # 16 row tiles per core
NO = D // FREE  # 4 output banks
XC = 128  # s-columns per packed x chunk (one GEMM row-tile)
NXC = S // XC  # 16 packed x chunks

F32 = mybir.dt.float32
BF16 = mybir.dt.bfloat16
BF_NP = ml_dtypes.bfloat16

def build_nc():
    nc = bacc.Bacc(
        "TRN2", target_bir_lowering=False, debug=False, num_devices=N_CORES
    )
    xp_d = nc.dram_tensor("xp", [NXC * P, ND * XC], BF16, kind="ExternalInput").ap()
    Wp_d = nc.dram_tensor("Wp", [NO * P, ND * FREE], BF16, kind="ExternalInput").ap()
    b_d = nc.dram_tensor("b", [D], F32, kind="ExternalInput").ap()
    BTs_d = nc.dram_tensor("BTs", [RANK, D], BF16, kind="ExternalInput").ap()
    A_d = nc.dram_tensor("A", [RANK, D], BF16, kind="ExternalInput").ap()
    out_d = nc.dram_tensor("out", [S, D], F32, kind="ExternalOutput").ap()

    with tile.TileContext(nc) as tc:
        with (
            tc.tile_pool(name="singles", bufs=1) as singles,
            tc.tile_pool(name="yout", bufs=4) as ypool,
            tc.tile_pool(name="dpsum", bufs=4, space="PSUM") as dpsum,
            tc.tile_pool(name="gpsum", bufs=4, space="PSUM") as gpsum,
        ):
            # A / 2B.T replicated into four 32-row bands (rows 32g..32g+15,
            # rest zero) so four rank-16 delta matmuls can run concurrently
            # in the four 32-row PE groups via tile_position
            A_sb = singles.tile([P, D], BF16)
            BTs_sb = singles.tile([P, D], BF16)
            bb = singles.tile([P, D], F32)
            jk = singles.tile([P, FREE], BF16)
            # resident operands, chunk-major to match the host packing
            wq = singles.tile([P, NO, ND, FREE], BF16)
            xT = singles.tile([P, NXC, ND, XC], BF16)

            nc.vector.memset(jk[:], 0.0)
            nc.vector.memset(A_sb[:], 0.0)
            nc.vector.memset(BTs_sb[:], 0.0)

            # ---- load schedule (sync ring; program order = drain order)
            nc.sync.dma_start(out=A_sb[0:RANK, :], in_=A_d[:])
            nc.sync.dma_start(out=BTs_sb[0:RANK, :], in_=BTs_d[:])
            # replicate into bands 1..3 on the (otherwise idle) GpSimd
            # engine — SBUF-to-SBUF only, no PSUM port needed
            for g in range(1, 4):
                nc.gpsimd.tensor_copy(
                    A_sb[32 * g : 32 * g + RANK, :], A_sb[0:RANK, :]
                )
                nc.gpsimd.tensor_copy(
                    BTs_sb[32 * g : 32 * g + RANK, :], BTs_sb[0:RANK, :]
                )

            def load_wt(ob, dg_lo=0, dg_hi=ND):
                nc.sync.dma_start(
                    out=wq[:, ob, dg_lo:dg_hi, :],
                    in_=Wp_d[
                        ob * P : (ob + 1) * P, dg_lo * FREE : dg_hi * FREE
                    ].rearrange("p (t o) -> p t o", t=dg_hi - dg_lo),
                )

            def load_x(c):
                nc.sync.dma_start(
                    out=xT[:, c, :, :],
                    in_=xp_d[c * P : (c + 1) * P, :].rearrange(
                        "p (t s) -> p t s", t=ND
                    ),
                )

            for dg in range(4):  # Wt bank 0 in four sub-chunks
                load_wt(0, 4 * dg, 4 * (dg + 1))
            load_x(0)
            load_x(1)
            nc.sync.dma_start(out=bb[:], in_=b_d[None, :].broadcast_to([P, D]))
            load_x(2)
            load_x(3)
            load_x(4)
            load_x(5)
            load_wt(1)
            for c in range(6, 11):
                load_x(c)
            load_wt(2)
            for c in range(11, 16):
                load_x(c)
            load_wt(3)

            _jn = [0]

            def junk_mm():
                # throwaway matmul: keeps the PE activity monitor warm
                _jn[0] += 1
                jp = gpsum.tile([P, FREE], F32, tag="gp", name=f"jp{_jn[0]}")
                nc.tensor.matmul(jp[:], jk[:, 0:P], jk[:], start=True, stop=True)

            stg = singles.tile([P, ND, FREE], BF16)

            def delta_cluster(ob, dt0, twophase=False):
                # wq[:, ob, dt0+g, :] += A[:, dblk].T @ (2*B.T)[:, ob-bank]
                # four rank-16 (zero-padded to K=32) deltas run concurrently
                # in the four 32-row PE groups.  twophase: ACT evicts
                # PSUM->bf16 staging, DVE adds all-bf16 at 2x — splits the
                # bank-0 merge wave across two engines.
                dps = []
                for g in range(4):
                    dt = dt0 + g
                    dp = dpsum.tile([P, FREE], F32, tag="dp", name=f"dp{ob}_{dt}")
                    nc.tensor.matmul(
                        dp[:],
                        A_sb[32 * g : 32 * (g + 1), dt * P : (dt + 1) * P],
                        BTs_sb[32 * g : 32 * (g + 1), ob * FREE : (ob + 1) * FREE],
                        start=True,
                        stop=True,
                        tile_position=(32 * g, 0),
                    )
                    dps.append(dp)
                for g in range(4):
                    dt = dt0 + g
                    sl = wq[:, ob, dt, :]
                    if twophase:
                        nc.scalar.copy(stg[:, dt, :], dps[g][:])
                        eng = nc.gpsimd if g % 2 else nc.vector
                        eng.tensor_add(sl, stg[:, dt, :], sl)
                    else:
                        nc.vector.tensor_add(sl, dps[g][:], sl)

            # PE warm-up while the first loads land
            for _ in range(6):
                junk_mm()
            # delta+merge for bank 0, junk-padded (merges chase the Wt0
            # sub-chunk DMAs and the ACT/DVE adds; junk keeps the PE
            # dense so HAM stays at full clock)
            for cl in range(4):
                delta_cluster(0, 4 * cl, twophase=True)
                for _ in range(4):
                    junk_mm()
            for _ in range(8):
                junk_mm()

            def lhs(st, dt):
                return xT[:, st, dt, :]

            # ---- main GEMM, ob-major ----
            for ob in range(NO):
                for st in range(NS):
                    if ob == 0 and 1 <= st <= 3:
                        junk_mm()
                        junk_mm()
                    gp = gpsum.tile([P, FREE], F32, tag="gp", name=f"gp{ob}_{st}")
                    for dt in range(ND):
                        nc.tensor.matmul(
                            gp[:],
                            lhs(st, dt),
                            wq[:, ob, dt, :],
                            start=(dt == 0),
                            stop=(dt == ND - 1),
                        )
                    if ob < NO - 1 and st in (8, 10, 12, 14):
                        delta_cluster(ob + 1, 2 * (st - 8), twophase=True)
                    yo = ypool.tile([P, FREE], F32, tag="yo", name=f"yo{ob}_{st}")
                    nc.vector.tensor_add(
                        yo[:], gp[:], bb[:, ob * FREE : (ob + 1) * FREE]
                    )
                    nc.scalar.dma_start(
                        out=out_d[
                            st * P : (st + 1) * P, ob * FREE : (ob + 1) * FREE
                        ],
                        in_=yo[:],
                    )

    nc.compile()
    return nc


_NC_CACHE = None


def _get_nc():
    global _NC_CACHE
    if _NC_CACHE is None:
        _NC_CACHE = build_nc()
    return _NC_CACHE


def make_in_maps(x, W, b, B, A):
    x = np.asarray(x, dtype=np.float32)
    W = np.asarray(W, dtype=np.float32)
    b = np.ascontiguousarray(b, dtype=np.float32)
    B = np.asarray(B, dtype=np.float32)
    A = np.asarray(A, dtype=np.float32)
    # xp[i, c, p, t, sc] = xT[i, t*128+p, c*256+sc] = x[i, c*256+sc, t*128+p]
    xT = np.ascontiguousarray(x.transpose(0, 2, 1)).astype(BF_NP)
    xp = np.ascontiguousarray(
        xT.reshape(BATCH, ND, P, NXC, XC).transpose(0, 3, 2, 1, 4)
    ).reshape(BATCH, NXC * P, ND * XC)
    # Wp[ob, p, t, oc] = W.T[t*128+p, ob*512+oc] = W[ob*512+oc, t*128+p]
    Wt = np.ascontiguousarray(W.T).astype(BF_NP)
    Wp = np.ascontiguousarray(
        Wt.reshape(ND, P, NO, FREE).transpose(2, 1, 0, 3)
    ).reshape(NO * P, ND * FREE)
    BTs = np.ascontiguousarray(SCALE * B.T).astype(BF_NP)
    Ab = A.astype(BF_NP)
    return [
        {"xp": xp[i], "Wp": Wp, "b": b, "BTs": BTs, "A": Ab}
        for i in range(N_CORES)
    ]


def run(inputs, **spmd_kwargs):
    """Run the SPMD kernel; returns (output, BassKernelResults)."""
    nc = _get_nc()
    in_maps = make_in_maps(**inputs)
    res = run_bass_kernel_spmd(nc, in_maps, core_ids=list(range(N_CORES)), **spmd_kwargs)
    out = np.stack([res.results[i]["out"] for i in range(N_CORES)]).astype(np.float32)
    return out, res


def kernel(x, W, b, B, A):
    out, _ = run({"x": x, "W": W, "b": b, "B": B, "A": A})
    return out


# revision 18
# speedup vs baseline: 1.1561x; 1.1561x over previous
"""Trainium2 Bass kernel for LoRA linear: y = x @ (W + 2*B@A).T + b.

Full inputs: x (8, 2048, 2048) f32, W (2048, 2048) f32, b (2048,) f32,
B (2048, 16) f32, A (16, 2048) f32.  Output (8, 2048, 2048) f32.

Sharding: data-parallel over the batch dim — core i computes
y[i] = x[i] @ w.T + b with the merged weight w = W + 2*B@A.

Host-side layout prep (sharding/packing only, no math): inputs are
pre-transposed, pre-cast to bf16, and pre-tiled into the exact SBUF
layouts the device wants, so every DMA is 128 fat descriptors (HWDGE
descriptor generation was the load bottleneck at ~3ns/descriptor):
  xp[c, p, t, sc] = x[c*256+sc, t*128+p]   (s-chunk-major tiles)
  Wp[ob, p, t, oc] = W[ob*512+oc, t*128+p] (o-bank-major tiles)
  BTs = 2*B.T (exact power-of-two scale; bf16 values identical to a
  device-side scale), A cast to bf16.

Device schedule (per core), tuned from perfetto traces:
  - all loads on ONE HWDGE ring (sync) in consumption-priority order
    (the order IS the prefetch schedule); stores on the other (scalar).
  - A and 2B.T land in zero-memset [128, D] tiles so the rank-16 delta
    matmuls are full-K=128 matmuls — identical shape to the GEMM MMs
    (K is free on the PE; K=16 stationaries cost ~+100ns transitions).
  - bank-0 delta merges are two-phase (ACT evicts PSUM to a bf16
    staging tile, DVE adds all-bf16 at 2x rate) so the head merge wave
    is split across two engines instead of serialized on the DVE.
  - throwaway warm-up matmuls keep the PE activity monitor from
    re-throttling the clock during the DMA/DVE-paced head (HAM drops
    the PE to 1.2 GHz after ~3.4us of low activity density).
  - main GEMM is ob-major: per output bank, 16 row-tiles of 16
    accumulating [128,128]x[128,512] bf16 matmuls; DVE adds the bias
    during PSUM->SBUF eviction.  Delta matmuls for bank ob+1 are
    spread two-per-group through the second half of pass ob so the PE
    stream never develops idle clusters.
"""

import numpy as np
import ml_dtypes

import concourse.bacc as bacc
import concourse.mybir as mybir
import concourse.tile as tile
from concourse.bass_utils import run_bass_kernel_spmd

N_CORES = 8
BATCH, S, D = 8, 2048, 2048
RANK = 16
SCALE = 2.0  # alpha / rank = 32 / 16
P = 128  # partitions
FREE = 512  # f32 elems per PSUM bank
ND = D // P  # 16 contraction tiles
NS = S // P  # 16 row tiles per core
NO = D // FREE  # 4 output banks
XC = 128  # s-columns per packed x chunk (one GEMM row-tile)
NXC = S // XC  # 16 packed x chunks

F32 = mybir.dt.float32
BF16 = mybir.dt.bfloat16
BF_NP = ml_dtypes.bfloat16

def build_nc():
    nc = bacc.Bacc(
        "TRN2", target_bir_lowering=False, debug=False, num_devices=N_CORES
    )
    xp_d = nc.dram_tensor("xp", [NXC * P, ND * XC], BF16, kind="ExternalInput").ap()
    Wp_d = nc.dram_tensor("Wp", [NO * P, ND * FREE], BF16, kind="ExternalInput").ap()
    b_d = nc.dram_tensor("b", [D], F32, kind="ExternalInput").ap()
    BTs_d = nc.dram_tensor("BTs", [RANK, D], BF16, kind="ExternalInput").ap()
    A_d = nc.dram_tensor("A", [RANK, D], BF16, kind="ExternalInput").ap()
    out_d = nc.dram_tensor("out", [S, D], F32, kind="ExternalOutput").ap()

    with tile.TileContext(nc) as tc:
        with (
            tc.tile_pool(name="singles", bufs=1) as singles,
            tc.tile_pool(name="yout", bufs=4) as ypool,
            tc.tile_pool(name="dpsum", bufs=1, space="PSUM") as dpsum,
            tc.tile_pool(name="gpsum", bufs=4, space="PSUM") as gpsum,
        ):
            # A / 2B.T replicated into four 32-row bands (rows 32g..32g+15,
            # rest zero) so four rank-16 delta matmuls can run concurrently
            # in the four 32-row PE groups via tile_position
            A_sb = singles.tile([P, D], BF16)
            BTs_sb = singles.tile([P, D], BF16)
            bb = singles.tile([P, D], F32)
            jk = singles.tile([P, FREE], BF16)
            # resident operands, chunk-major to match the host packing
            wq = singles.tile([P, NO, ND, FREE], BF16)
            xT = singles.tile([P, NXC, ND, XC], BF16)

            nc.vector.memset(jk[:], 0.0)
            nc.vector.memset(A_sb[:], 0.0)
            nc.vector.memset(BTs_sb[:], 0.0)

            # ---- load schedule (sync ring; program order = drain order)
            for g in range(4):
                nc.sync.dma_start(out=A_sb[32 * g : 32 * g + RANK, :], in_=A_d[:])
                nc.sync.dma_start(
                    out=BTs_sb[32 * g : 32 * g + RANK, :], in_=BTs_d[:]
                )

            def load_wt(ob, dg_lo=0, dg_hi=ND):
                nc.sync.dma_start(
                    out=wq[:, ob, dg_lo:dg_hi, :],
                    in_=Wp_d[
                        ob * P : (ob + 1) * P, dg_lo * FREE : dg_hi * FREE
                    ].rearrange("p (t o) -> p t o", t=dg_hi - dg_lo),
                )

            def load_x(c):
                nc.sync.dma_start(
                    out=xT[:, c, :, :],
                    in_=xp_d[c * P : (c + 1) * P, :].rearrange(
                        "p (t s) -> p t s", t=ND
                    ),
                )

            for dg in range(4):  # Wt bank 0 in four sub-chunks
                load_wt(0, 4 * dg, 4 * (dg + 1))
            load_x(0)
            load_x(1)
            nc.sync.dma_start(out=bb[:], in_=b_d[None, :].broadcast_to([P, D]))
            load_x(2)
            load_x(3)
            load_x(4)
            load_x(5)
            load_wt(1)
            for c in range(6, 11):
                load_x(c)
            load_wt(2)
            for c in range(11, 16):
                load_x(c)
            load_wt(3)

            _jn = [0]

            def junk_mm():
                # throwaway matmul: keeps the PE activity monitor warm
                _jn[0] += 1
                jp = gpsum.tile([P, FREE], F32, tag="gp", name=f"jp{_jn[0]}")
                nc.tensor.matmul(jp[:], jk[:, 0:P], jk[:], start=True, stop=True)

            stg = singles.tile([P, ND, FREE], BF16)

            def delta_cluster(ob, dt0, twophase=False):
                # wq[:, ob, dt0+g, :] += A[:, dblk].T @ (2*B.T)[:, ob-bank]
                # four rank-16 (zero-padded to K=32) deltas run concurrently
                # in the four 32-row PE groups, into the four banks of ONE
                # psum tile.  twophase: a single 4-bank-wide ACT eviction to
                # bf16 staging + a single 4-wide DVE bf16 add (fewer per-op
                # overheads than 16 narrow ops, split across two engines).
                dp = dpsum.tile([P, 4, FREE], F32, tag="dp", name=f"dp{ob}_{dt0}")
                for g in range(4):
                    dt = dt0 + g
                    nc.tensor.matmul(
                        dp[:, g, :],
                        A_sb[32 * g : 32 * (g + 1), dt * P : (dt + 1) * P],
                        BTs_sb[32 * g : 32 * (g + 1), ob * FREE : (ob + 1) * FREE],
                        start=True,
                        stop=True,
                        tile_position=(32 * g, 0),
                    )
                sl = wq[:, ob, dt0 : dt0 + 4, :]
                if twophase:
                    nc.scalar.copy(stg[:, dt0 : dt0 + 4, :], dp[:])
                    nc.vector.tensor_add(sl, stg[:, dt0 : dt0 + 4, :], sl)
                else:
                    nc.vector.tensor_add(sl, dp[:], sl)

            # PE warm-up while the first loads land
            for _ in range(6):
                junk_mm()
            # delta+merge for bank 0, junk-padded (merges chase the Wt0
            # sub-chunk DMAs and the ACT/DVE adds; junk keeps the PE
            # dense so HAM stays at full clock)
            for cl in range(4):
                delta_cluster(0, 4 * cl, twophase=True)
                for _ in range(4):
                    junk_mm()
            for _ in range(8):
                junk_mm()

            def lhs(st, dt):
                return xT[:, st, dt, :]

            # ---- main GEMM, ob-major ----
            for ob in range(NO):
                for st in range(NS):
                    if ob == 0 and 1 <= st <= 3:
                        junk_mm()
                        junk_mm()
                    gp = gpsum.tile([P, FREE], F32, tag="gp", name=f"gp{ob}_{st}")
                    for dt in range(ND):
                        nc.tensor.matmul(
                            gp[:],
                            lhs(st, dt),
                            wq[:, ob, dt, :],
                            start=(dt == 0),
                            stop=(dt == ND - 1),
                        )
                    if ob < NO - 1 and st in (8, 10, 12, 14):
                        delta_cluster(ob + 1, 2 * (st - 8), twophase=True)
                    yo = ypool.tile([P, FREE], F32, tag="yo", name=f"yo{ob}_{st}")
                    nc.vector.tensor_add(
                        yo[:], gp[:], bb[:, ob * FREE : (ob + 1) * FREE]
                    )
                    nc.scalar.dma_start(
                        out=out_d[
                            st * P : (st + 1) * P, ob * FREE : (ob + 1) * FREE
                        ],
                        in_=yo[:],
                    )

    nc.compile()
    return nc


_NC_CACHE = None


def _get_nc():
    global _NC_CACHE
    if _NC_CACHE is None:
        _NC_CACHE = build_nc()
    return _NC_CACHE


def make_in_maps(x, W, b, B, A):
    x = np.asarray(x, dtype=np.float32)
    W = np.asarray(W, dtype=np.float32)
    b = np.ascontiguousarray(b, dtype=np.float32)
    B = np.asarray(B, dtype=np.float32)
    A = np.asarray(A, dtype=np.float32)
    # xp[i, c, p, t, sc] = xT[i, t*128+p, c*256+sc] = x[i, c*256+sc, t*128+p]
    xT = np.ascontiguousarray(x.transpose(0, 2, 1)).astype(BF_NP)
    xp = np.ascontiguousarray(
        xT.reshape(BATCH, ND, P, NXC, XC).transpose(0, 3, 2, 1, 4)
    ).reshape(BATCH, NXC * P, ND * XC)
    # Wp[ob, p, t, oc] = W.T[t*128+p, ob*512+oc] = W[ob*512+oc, t*128+p]
    Wt = np.ascontiguousarray(W.T).astype(BF_NP)
    Wp = np.ascontiguousarray(
        Wt.reshape(ND, P, NO, FREE).transpose(2, 1, 0, 3)
    ).reshape(NO * P, ND * FREE)
    BTs = np.ascontiguousarray(SCALE * B.T).astype(BF_NP)
    Ab = A.astype(BF_NP)
    return [
        {"xp": xp[i], "Wp": Wp, "b": b, "BTs": BTs, "A": Ab}
        for i in range(N_CORES)
    ]


def run(inputs, **spmd_kwargs):
    """Run the SPMD kernel; returns (output, BassKernelResults)."""
    nc = _get_nc()
    in_maps = make_in_maps(**inputs)
    res = run_bass_kernel_spmd(nc, in_maps, core_ids=list(range(N_CORES)), **spmd_kwargs)
    out = np.stack([res.results[i]["out"] for i in range(N_CORES)]).astype(np.float32)
    return out, res


def kernel(x, W, b, B, A):
    out, _ = run({"x": x, "W": W, "b": b, "B": B, "A": A})
    return out


# revision 19
# speedup vs baseline: 1.1645x; 1.0073x over previous
"""Trainium2 Bass kernel for LoRA linear: y = x @ (W + 2*B@A).T + b.

Full inputs: x (8, 2048, 2048) f32, W (2048, 2048) f32, b (2048,) f32,
B (2048, 16) f32, A (16, 2048) f32.  Output (8, 2048, 2048) f32.

Sharding: data-parallel over the batch dim — core i computes
y[i] = x[i] @ w.T + b with the merged weight w = W + 2*B@A.

Host-side layout prep (sharding/packing only, no math): inputs are
pre-transposed, pre-cast to bf16, and pre-tiled into the exact SBUF
layouts the device wants, so every DMA is 128 fat descriptors (HWDGE
descriptor generation was the load bottleneck at ~3ns/descriptor):
  xp[c, p, t, sc] = x[c*256+sc, t*128+p]   (s-chunk-major tiles)
  Wp[ob, p, t, oc] = W[ob*512+oc, t*128+p] (o-bank-major tiles)
  BTs = 2*B.T (exact power-of-two scale; bf16 values identical to a
  device-side scale), A cast to bf16.

Device schedule (per core), tuned from perfetto traces:
  - all loads on ONE HWDGE ring (sync) in consumption-priority order
    (the order IS the prefetch schedule); stores on the other (scalar).
  - A and 2B.T land in zero-memset [128, D] tiles so the rank-16 delta
    matmuls are full-K=128 matmuls — identical shape to the GEMM MMs
    (K is free on the PE; K=16 stationaries cost ~+100ns transitions).
  - bank-0 delta merges are two-phase (ACT evicts PSUM to a bf16
    staging tile, DVE adds all-bf16 at 2x rate) so the head merge wave
    is split across two engines instead of serialized on the DVE.
  - throwaway warm-up matmuls keep the PE activity monitor from
    re-throttling the clock during the DMA/DVE-paced head (HAM drops
    the PE to 1.2 GHz after ~3.4us of low activity density).
  - main GEMM is ob-major: per output bank, 16 row-tiles of 16
    accumulating [128,128]x[128,512] bf16 matmuls; DVE adds the bias
    during PSUM->SBUF eviction.  Delta matmuls for bank ob+1 are
    spread two-per-group through the second half of pass ob so the PE
    stream never develops idle clusters.
"""

import numpy as np
import ml_dtypes

import concourse.bacc as bacc
import concourse.mybir as mybir
import concourse.tile as tile
from concourse.bass_utils import run_bass_kernel_spmd

N_CORES = 8
BATCH, S, D = 8, 2048, 2048
RANK = 16
SCALE = 2.0  # alpha / rank = 32 / 16
P = 128  # partitions
FREE = 512  # f32 elems per PSUM bank
ND = D // P  # 16 contraction tiles
NS = S // P  # 16 row tiles per core
NO = D // FREE  # 4 output banks
XC = 128  # s-columns per packed x chunk (one GEMM row-tile)
NXC = S // XC  # 16 packed x chunks

F32 = mybir.dt.float32
BF16 = mybir.dt.bfloat16
BF_NP = ml_dtypes.bfloat16

def build_nc():
    nc = bacc.Bacc(
        "TRN2", target_bir_lowering=False, debug=False, num_devices=N_CORES
    )
    xp_d = nc.dram_tensor("xp", [NXC * P, ND * XC], BF16, kind="ExternalInput").ap()
    Wp_d = nc.dram_tensor("Wp", [NO * P, ND * FREE], BF16, kind="ExternalInput").ap()
    b_d = nc.dram_tensor("b", [D], F32, kind="ExternalInput").ap()
    BTs_d = nc.dram_tensor("BTs", [RANK, D], BF16, kind="ExternalInput").ap()
    A_d = nc.dram_tensor("A", [RANK, D], BF16, kind="ExternalInput").ap()
    out_d = nc.dram_tensor("out", [S, D], F32, kind="ExternalOutput").ap()

    with tile.TileContext(nc) as tc:
        with (
            tc.tile_pool(name="singles", bufs=1) as singles,
            tc.tile_pool(name="yout", bufs=4) as ypool,
            tc.tile_pool(name="dpsum", bufs=1, space="PSUM") as dpsum,
            tc.tile_pool(name="gpsum", bufs=4, space="PSUM") as gpsum,
        ):
            # A / 2B.T replicated into four 32-row bands (rows 32g..32g+15,
            # rest zero) so four rank-16 delta matmuls can run concurrently
            # in the four 32-row PE groups via tile_position
            A_sb = singles.tile([P, D], BF16)
            BTs_sb = singles.tile([P, D], BF16)
            bb = singles.tile([P, D], F32)
            jk = singles.tile([P, FREE], BF16)
            # resident operands, chunk-major to match the host packing
            wq = singles.tile([P, NO, ND, FREE], BF16)
            xT = singles.tile([P, NXC, ND, XC], BF16)

            nc.vector.memset(jk[:], 0.0)
            nc.vector.memset(A_sb[:], 0.0)
            nc.vector.memset(BTs_sb[:], 0.0)

            # ---- load schedule (sync ring; program order = drain order)
            for g in range(4):
                nc.sync.dma_start(out=A_sb[32 * g : 32 * g + RANK, :], in_=A_d[:])
                nc.sync.dma_start(
                    out=BTs_sb[32 * g : 32 * g + RANK, :], in_=BTs_d[:]
                )

            def load_wt(ob, dg_lo=0, dg_hi=ND):
                nc.sync.dma_start(
                    out=wq[:, ob, dg_lo:dg_hi, :],
                    in_=Wp_d[
                        ob * P : (ob + 1) * P, dg_lo * FREE : dg_hi * FREE
                    ].rearrange("p (t o) -> p t o", t=dg_hi - dg_lo),
                )

            def load_x(c):
                nc.sync.dma_start(
                    out=xT[:, c, :, :],
                    in_=xp_d[c * P : (c + 1) * P, :].rearrange(
                        "p (t s) -> p t s", t=ND
                    ),
                )

            # Wt bank 0 sub-chunks interleaved with the first x chunks so
            # row-tile 0's merge-chase overlaps the remaining loads
            load_x(0)
            load_wt(0, 0, 4)
            load_wt(0, 4, 8)
            load_x(1)
            load_wt(0, 8, 12)
            load_wt(0, 12, 16)
            nc.sync.dma_start(out=bb[:], in_=b_d[None, :].broadcast_to([P, D]))
            load_x(2)
            load_x(3)
            load_x(4)
            load_x(5)
            load_wt(1)
            for c in range(6, 11):
                load_x(c)
            load_wt(2)
            for c in range(11, 16):
                load_x(c)
            load_wt(3)

            _jn = [0]

            def junk_mm():
                # throwaway matmul: keeps the PE activity monitor warm
                _jn[0] += 1
                jp = gpsum.tile([P, FREE], F32, tag="gp", name=f"jp{_jn[0]}")
                nc.tensor.matmul(jp[:], jk[:, 0:P], jk[:], start=True, stop=True)

            stg = singles.tile([P, ND, FREE], BF16)

            def delta_cluster(ob, dt0, twophase=False):
                # wq[:, ob, dt0+g, :] += A[:, dblk].T @ (2*B.T)[:, ob-bank]
                # four rank-16 (zero-padded to K=32) deltas run concurrently
                # in the four 32-row PE groups, into the four banks of ONE
                # psum tile.  twophase: a single 4-bank-wide ACT eviction to
                # bf16 staging + a single 4-wide DVE bf16 add (fewer per-op
                # overheads than 16 narrow ops, split across two engines).
                dp = dpsum.tile([P, 4, FREE], F32, tag="dp", name=f"dp{ob}_{dt0}")
                for g in range(4):
                    dt = dt0 + g
                    nc.tensor.matmul(
                        dp[:, g, :],
                        A_sb[32 * g : 32 * (g + 1), dt * P : (dt + 1) * P],
                        BTs_sb[32 * g : 32 * (g + 1), ob * FREE : (ob + 1) * FREE],
                        start=True,
                        stop=True,
                        tile_position=(32 * g, 0),
                    )
                sl = wq[:, ob, dt0 : dt0 + 4, :]
                if twophase:
                    nc.scalar.copy(stg[:, dt0 : dt0 + 4, :], dp[:])
                    nc.vector.tensor_add(sl, stg[:, dt0 : dt0 + 4, :], sl)
                else:
                    nc.vector.tensor_add(sl, dp[:], sl)

            # PE warm-up while the first loads land
            for _ in range(6):
                junk_mm()
            # delta+merge for bank 0, junk-padded (merges chase the Wt0
            # sub-chunk DMAs and the ACT/DVE adds; junk keeps the PE
            # dense so HAM stays at full clock)
            for cl in range(4):
                delta_cluster(0, 4 * cl, twophase=True)
                for _ in range(4):
                    junk_mm()
            for _ in range(14):
                junk_mm()

            def lhs(st, dt):
                return xT[:, st, dt, :]

            # ---- main GEMM, ob-major ----
            for ob in range(NO):
                for st in range(NS):
                    if ob == 0 and 1 <= st <= 3:
                        junk_mm()
                        junk_mm()
                    gp = gpsum.tile([P, FREE], F32, tag="gp", name=f"gp{ob}_{st}")
                    for dt in range(ND):
                        nc.tensor.matmul(
                            gp[:],
                            lhs(st, dt),
                            wq[:, ob, dt, :],
                            start=(dt == 0),
                            stop=(dt == ND - 1),
                        )
                    if ob < NO - 1 and st in (8, 10, 12, 14):
                        delta_cluster(ob + 1, 2 * (st - 8), twophase=True)
                    yo = ypool.tile([P, FREE], F32, tag="yo", name=f"yo{ob}_{st}")
                    nc.vector.tensor_add(
                        yo[:], gp[:], bb[:, ob * FREE : (ob + 1) * FREE]
                    )
                    nc.scalar.dma_start(
                        out=out_d[
                            st * P : (st + 1) * P, ob * FREE : (ob + 1) * FREE
                        ],
                        in_=yo[:],
                    )

    nc.compile()
    return nc


_NC_CACHE = None


def _get_nc():
    global _NC_CACHE
    if _NC_CACHE is None:
        _NC_CACHE = build_nc()
    return _NC_CACHE


def make_in_maps(x, W, b, B, A):
    x = np.asarray(x, dtype=np.float32)
    W = np.asarray(W, dtype=np.float32)
    b = np.ascontiguousarray(b, dtype=np.float32)
    B = np.asarray(B, dtype=np.float32)
    A = np.asarray(A, dtype=np.float32)
    # xp[i, c, p, t, sc] = xT[i, t*128+p, c*256+sc] = x[i, c*256+sc, t*128+p]
    xT = np.ascontiguousarray(x.transpose(0, 2, 1)).astype(BF_NP)
    xp = np.ascontiguousarray(
        xT.reshape(BATCH, ND, P, NXC, XC).transpose(0, 3, 2, 1, 4)
    ).reshape(BATCH, NXC * P, ND * XC)
    # Wp[ob, p, t, oc] = W.T[t*128+p, ob*512+oc] = W[ob*512+oc, t*128+p]
    Wt = np.ascontiguousarray(W.T).astype(BF_NP)
    Wp = np.ascontiguousarray(
        Wt.reshape(ND, P, NO, FREE).transpose(2, 1, 0, 3)
    ).reshape(NO * P, ND * FREE)
    BTs = np.ascontiguousarray(SCALE * B.T).astype(BF_NP)
    Ab = A.astype(BF_NP)
    return [
        {"xp": xp[i], "Wp": Wp, "b": b, "BTs": BTs, "A": Ab}
        for i in range(N_CORES)
    ]


def run(inputs, **spmd_kwargs):
    """Run the SPMD kernel; returns (output, BassKernelResults)."""
    nc = _get_nc()
    in_maps = make_in_maps(**inputs)
    res = run_bass_kernel_spmd(nc, in_maps, core_ids=list(range(N_CORES)), **spmd_kwargs)
    out = np.stack([res.results[i]["out"] for i in range(N_CORES)]).astype(np.float32)
    return out, res


def kernel(x, W, b, B, A):
    out, _ = run({"x": x, "W": W, "b": b, "B": B, "A": A})
    return out
